# revision 11
# baseline (speedup 1.0000x reference)
"""KMeansQuantizer Trainium2 kernel, v2.

reference: idx[b,t] = argmin_k ||x[b,t] - c_k||^2 over K=2048 centroids
         = argmax_k ( x.c_k - ||c_k||^2/2 )

Data-parallel over 8 NeuronCores (4096 rows each, padded from 32000).

Device program is pure matmul + argmax — all layout work is hoisted to the
host so the PE does nothing but roofline f32r matmuls:
  - x is pre-transposed on the host to [E, rows] per core, so the per-tile
    PE transposes of the baseline (and their PSUM traffic + ACT drains) are
    gone entirely. The gpsimd (SWDGE) DMA casts f32 -> f32r in flight.
  - centroids are pre-transposed to [E, K] on the host and land via both
    HWDGE queues into f32 stage tiles; one ACT copy per 128-row chunk
    converts to the resident f32r codebook (no on-device transpose pass).
  - the -||c||^2/2 bias row is computed on the host and shipped replicated
    to all 128 partitions ([128, K] input): no on-device norms/replication.

Per 128-row tile: 8 stationary loads (one per e-chunk), 4 psum banks of
512 k-columns each, 32 f32r matmuls (1 cyc/row at ap>=256, 213ns each)
= 6.83us PE at 2.4GHz. DVE drains the 4 banks with tensor_add (+bias),
then max8 over the full 2048-wide dist row (true top-2 values) and a
single max_index scan: ~6.8us DVE, just under the PE time (the two small
output staging copies ride gpsimd to keep it there). PSUM uses all 8
banks (4 per tile, double-buffered across tiles), so matmuls of tile t+1
overlap the DVE drain of tile t.

Measured HW (marginal method, R=33 reps in one NEFF, median of 16
pairs): ~265us/pass vs the baseline's ~300us pass1 + ~97us repair pass.
The f32r PE roofline is 218.6us (32 tiles x 16384 moving-column cycles
at 2.4GHz); the remainder is ~1.13x effective PE pacing (stationary
reloads) plus the per-rep codebook reload. Variants that move the PSUM
drain off DVE (ACT psum->sbuf copy, or ACT bias-preload + DVE max direct
from PSUM) measured slower (278us / 314us) — PE is the binding engine.
NB: tensor_tensor_reduce (fused add+max drain) hard-crashes the exec
unit (NRT_EXEC_UNIT_UNRECOVERABLE) on this toolchain; the SWDGE cast-DMA
x-path costs ~22us over HWDGE+ACT-convert. Those paths are kept behind
build flags for reference but must stay off.

Accuracy: f32r matmul error is ~0.008 absolute on scores whose typical
top-2 gap is ~8, giving ~16 row flips per 32k rows (mean rel err 8.5e-4,
23x inside the 2e-2 gate) with no repair at all. We still repair: rows
whose top-2 gap is < THRESH are recomputed exactly on the host in fp64
(a few hundred rows, ~1 GFLOP of numpy) — measured 0 mismatches of
32000.
"""
import numpy as np

import concourse.bacc as bacc
import concourse.mybir as mybir
import concourse.tile as tile
from concourse.bass_utils import run_bass_kernel_spmd

B, T, E, K = 16, 2000, 1024, 2048
N_CORES = 8
N_ROWS = B * T                    # 32000
ROWS_PER_CORE = 4096              # padded total 32768
N_TILES = ROWS_PER_CORE // 128    # 32
EC = E // 128                     # 8 e-chunks
NBANK = 4                         # psum banks of 512 k-columns
OGROUP = 4                        # row tiles per output DMA
QCOLS = 1024                      # x columns (= rows of x) per streamed chunk
TPQ = QCOLS // 128                # row tiles per chunk
NQ = ROWS_PER_CORE // QCOLS       # 4 streamed chunks

THRESH = 0.075                    # half-bank top-2 gap below this -> host repair
NEG_INF = -3.0e38

F32 = mybir.dt.float32
F32R = mybir.dt.float32r
U32 = mybir.dt.uint32


def build(n_tiles=N_TILES, reps=1, use_ttr=False, cast_dma=False,
          psum_bufs=8, psum_direct=False, drain_act=False):
    """One NeuronCore program: xT [E, n_tiles*128] (transposed rows),
    cT [E, K], bias [128, K] -> per row argmax index + top-2 half-bank
    maxima, packed as f32 triples."""
    nc = bacc.Bacc("TRN2", target_bir_lowering=False, debug=False)

    rows = n_tiles * 128
    n_og = (n_tiles + OGROUP - 1) // OGROUP
    xT_d = nc.dram_tensor("x", [E, rows], F32, kind="ExternalInput")
    cT_d = nc.dram_tensor("c", [E, K], F32, kind="ExternalInput")
    b_d = nc.dram_tensor("b", [128, K], F32, kind="ExternalInput")
    out_d = nc.dram_tensor("out", [n_og, 128, 3 * OGROUP], F32,
                           kind="ExternalOutput")

    with tile.TileContext(nc) as tc:
        with (
            tc.tile_pool(name="const", bufs=1) as constp,
            tc.tile_pool(name="ctp", bufs=1) as ctp,
            tc.tile_pool(name="cstage", bufs=2) as cstage,
            tc.tile_pool(name="xq", bufs=2) as xqp,
            tc.tile_pool(name="dst", bufs=2) as dst,
            tc.tile_pool(name="mxp", bufs=3) as mxp,
            tc.tile_pool(name="og", bufs=2) as ogp,
            tc.tile_pool(name="psum", bufs=(2 if psum_direct else psum_bufs),
                         space="PSUM") as psum,
        ):
            for _rep in range(reps):
                # codebook: 8 e-chunks of [128, K] via both HWDGE queues into
                # f32 stage tiles, ACT-converted to resident f32r tiles
                cT = []
                for i in range(EC):
                    cst = cstage.tile([128, K], F32, tag="cstage",
                                      name=f"cst{i}")
                    ceng = nc.sync if i % 2 == 0 else nc.scalar
                    ceng.dma_start(cst, cT_d[i * 128:(i + 1) * 128, :])
                    cti = ctp.tile([128, K], F32R, tag=f"ct{i}", name=f"ct{i}")
                    nc.scalar.copy(cti, cst)
                    cT.append(cti)

                # bias rides sync behind the codebook (first needed by DVE,
                # well after the first matmuls)
                bias = constp.tile([128, K], F32, tag="bias", name="bias")
                nc.sync.dma_start(bias, b_d[:, :])

                # x streamed in [E, QCOLS] chunks (8 e-chunk tiles each) on
                # the SWDGE queue, double-buffered; the gpsimd DMA casts
                # f32 -> f32r in flight
                xq = {}

                def load_quarter(q):
                    if q >= (n_tiles + TPQ - 1) // TPQ:
                        return
                    tiles = []
                    for i in range(EC):
                        xt = xqp.tile([128, QCOLS], F32R, tag=f"xq{i}",
                                      name=f"xq{q}_{i}")
                        if cast_dma:
                            nc.gpsimd.dma_start(
                                xt, xT_d[i * 128:(i + 1) * 128,
                                         q * QCOLS:(q + 1) * QCOLS])
                        else:
                            # HWDGE (fast hardware DGE) + ACT f32->f32r
                            # convert; the SWDGE software queue throttles
                            # well below HWDGE on this stream
                            xst = cstage.tile([128, QCOLS], F32, tag="xstage",
                                              name=f"xs{q}_{i}")
                            xeng = nc.sync if (q + i) % 2 == 0 else nc.scalar
                            xeng.dma_start(
                                xst, xT_d[i * 128:(i + 1) * 128,
                                          q * QCOLS:(q + 1) * QCOLS])
                            nc.scalar.copy(xt, xst)
                        tiles.append(xt)
                    xq[q] = tiles

                load_quarter(0)
                load_quarter(1)

                ostg = {}
                for t in range(n_tiles):
                    q, r = divmod(t, TPQ)
                    if r == 0:
                        load_quarter(q + 2)

                    if psum_direct:
                        # one [128, K] psum tile = 4 banks; ACT preloads the
                        # bias so matmuls accumulate on top (start=False) and
                        # DVE can max over biased scores straight from PSUM
                        pst = psum.tile([128, K], F32, tag="pst",
                                        name=f"pst{t}")
                        for b in range(NBANK):
                            nc.scalar.copy(pst[:, b * 512:(b + 1) * 512],
                                           bias[:, b * 512:(b + 1) * 512])
                        ps = [pst[:, b * 512:(b + 1) * 512]
                              for b in range(NBANK)]
                    else:
                        ps = [psum.tile([128, 512], F32, tag="ps",
                                        name=f"ps{t}_{b}")
                              for b in range(NBANK)]
                    xt = xq[q]
                    for i in range(EC):
                        stat = xt[i][:, r * 128:(r + 1) * 128]
                        for b in range(NBANK):
                            nc.tensor.matmul(
                                ps[b], stat, cT[i][:, b * 512:(b + 1) * 512],
                                start=(False if psum_direct else i == 0),
                                stop=(i == EC - 1),
                                skip_group_check=psum_direct)
                    if r == TPQ - 1:
                        xq.pop(q, None)

                    # fused drain: dist = psum + bias, half-bank running max
                    m8 = mxp.tile([128, 8], F32, tag="m8", name=f"m8{t}")
                    mi = mxp.tile([128, 8], U32, tag="mi", name=f"mi{t}")
                    if psum_direct:
                        nc.vector.max(out=m8, in_=pst[:, :])
                        nc.vector.max_index(out=mi, in_max=m8,
                                            in_values=pst[:, :])
                    elif use_ttr:
                        dist = dst.tile([128, K], F32, tag="dist",
                                        name=f"dist{t}")
                        hmax = mxp.tile([128, 8], F32, tag="hmax",
                                        name=f"hmax{t}")
                        for h in range(8):
                            b, half = divmod(h, 2)
                            nc.vector.tensor_tensor_reduce(
                                out=dist[:, h * 256:(h + 1) * 256],
                                in0=ps[b][:, half * 256:(half + 1) * 256],
                                in1=bias[:, h * 256:(h + 1) * 256],
                                scale=1.0, scalar=NEG_INF,
                                op0=mybir.AluOpType.add,
                                op1=mybir.AluOpType.max,
                                accum_out=hmax[:, h:h + 1])
                        nc.vector.max(out=m8, in_=hmax)
                        nc.vector.max_index(out=mi, in_max=m8,
                                            in_values=dist)
                    elif drain_act:
                        # DVE reads PSUM at reduced rate on HW, so ACT (which
                        # has slack) drains the raw scores to SBUF and DVE
                        # adds the bias entirely in SBUF
                        draw = dst.tile([128, K], F32, tag="draw",
                                        name=f"draw{t}")
                        dist = dst.tile([128, K], F32, tag="dist",
                                        name=f"dist{t}")
                        for b in range(NBANK):
                            nc.scalar.copy(draw[:, b * 512:(b + 1) * 512],
                                           ps[b])
                        for h in range(2):
                            nc.vector.tensor_add(
                                dist[:, h * 1024:(h + 1) * 1024],
                                draw[:, h * 1024:(h + 1) * 1024],
                                bias[:, h * 1024:(h + 1) * 1024])
                        nc.vector.max(out=m8, in_=dist)
                        nc.vector.max_index(out=mi, in_max=m8,
                                            in_values=dist)
                    else:
                        dist = dst.tile([128, K], F32, tag="dist",
                                        name=f"dist{t}")
                        for b in range(NBANK):
                            nc.vector.tensor_add(
                                dist[:, b * 512:(b + 1) * 512], ps[b],
                                bias[:, b * 512:(b + 1) * 512])
                        nc.vector.max(out=m8, in_=dist)
                        nc.vector.max_index(out=mi, in_max=m8,
                                            in_values=dist)

                    g, rr = divmod(t, OGROUP)
                    if rr == 0:
                        ostg[g] = ogp.tile([128, 3 * OGROUP], F32, tag="ostg",
                                           name=f"ostg{g}")
                    # u32 idx -> f32 convert is exact (idx < 2^24);
                    # staging copies ride gpsimd to keep DVE under the PE time
                    nc.gpsimd.tensor_copy(ostg[g][:, 3 * rr:3 * rr + 1],
                                          mi[:, 0:1])
                    nc.gpsimd.tensor_copy(ostg[g][:, 3 * rr + 1:3 * rr + 3],
                                          m8[:, 0:2])
                    if rr == OGROUP - 1 or t == n_tiles - 1:
                        nc.sync.dma_start(out_d[g, :, :], ostg[g])
                        ostg.pop(g, None)

    nc.compile()
    return nc


_cache = {}


def _get_nc(key, **kw):
    if key not in _cache:
        _cache[key] = build(**kw)
    return _cache[key]


def make_in_maps(x_flat_padded, c):
    """x_flat_padded [N_CORES*ROWS_PER_CORE, E] f32, c [K, E] f32 ->
    per-core input dicts with host-transposed layouts."""
    xT = np.ascontiguousarray(
        x_flat_padded.reshape(N_CORES, ROWS_PER_CORE, E).transpose(0, 2, 1))
    cT = np.ascontiguousarray(c.T)
    nrm = np.einsum("ke,ke->k", c.astype(np.float64), c.astype(np.float64))
    bias = np.ascontiguousarray(
        np.broadcast_to((-0.5 * nrm).astype(np.float32)[None, :], (128, K)))
    return [{"x": xT[i], "c": cT, "b": bias} for i in range(N_CORES)], nrm


def _decode(res, n_tiles=N_TILES):
    idxs, vals = [], []
    for r in res.results:
        o = r["out"]                              # [n_og, 128, 3*OGROUP]
        n_og = o.shape[0]
        o = o.reshape(n_og, 128, OGROUP, 3).transpose(0, 2, 1, 3)
        o = o.reshape(n_og * OGROUP * 128, 3)[:n_tiles * 128]
        idxs.append(o[:, 0].astype(np.int64))
        vals.append(o[:, 1:3].astype(np.float32))
    return np.concatenate(idxs), np.concatenate(vals)


def kernel(x, centroids):
    x_flat = np.asarray(x, dtype=np.float32).reshape(N_ROWS, E)
    c = np.ascontiguousarray(np.asarray(centroids, dtype=np.float32))

    xp = np.zeros((N_CORES * ROWS_PER_CORE, E), dtype=np.float32)
    xp[:N_ROWS] = x_flat
    in_maps, nrm = make_in_maps(xp, c)

    nc = _get_nc(("p1",))
    res = run_bass_kernel_spmd(nc, in_maps, core_ids=list(range(N_CORES)))
    idx, val = _decode(res)
    idx = idx[:N_ROWS]

    # host repair: exact fp64 argmin for rows whose half-bank top-2 gap is
    # within the f32r error margin
    gap = val[:N_ROWS, 0] - val[:N_ROWS, 1]
    suspects = np.flatnonzero(gap < THRESH)
    if len(suspects):
        d = nrm[None, :] - 2.0 * (x_flat[suspects].astype(np.float64)
                                  @ c.T.astype(np.float64))
        idx = idx.copy()
        idx[suspects] = np.argmin(d, axis=1)
    return idx.reshape(B, T)
